# revision 27
# baseline (speedup 1.0000x reference)
"""Trainium2 Bass kernel for nn_BotAwareGAT (2-layer hetero GAT + MLP).

Strategy (8 NeuronCores, SPMD, dst-partitioned):
  - Core k owns dst nodes [k*2500, (k+1)*2500).
  - Node tables (one per edge type per layer) hold 512B rows:
    [h(128 bf16) | s_src(8 bf16) | s_dst(8 bf16) | pad].  Table builds are
    SHARDED: each core computes its 2500 rows, then AllGather fills the
    replicated table.  Layer-2 tables hold h1' (post-ELU layer-1 output),
    NOT h2: since out2 = W2^T (sum_e alpha_e h1'[src_e]) the W2 projection
    is linear and is applied per-dst AFTER aggregation, so edges only ever
    gather 512B rows (vs 1280B for h2-space rows).
  - Per-edge softmax-aggregation uses an ELL schedule (degree-sorted dst
    groups); messages fetched with batched dma_gather (2048 idx/call).
    L1 (GD=64): psum += S^T @ (q (*) h) per 128-edge chunk (S one-hot).
    L2 (GD=16): M_T[c1, h*16+d] += G_h1^T @ (q (*) S) per chunk; per
    128-dst batch M_T is normalized and projected through W2 per head.
  - Padding edges point at a poison row (s_src=-600 => q underflows to 0).
"""

import numpy as np
import ml_dtypes

N = 20000
NCORES = 8
ND = N // NCORES            # 2500 dst nodes per core
NSLOT = 2560                # padded slots per core
GD1 = 64                    # L1 dsts per group
GD2 = 16                    # L2 dsts per group
NG1 = NSLOT // GD1          # 40
NG2 = NSLOT // GD2          # 160
NCH = 8                     # chunks per dma_gather call (1024 idx)
POISON = N                  # poison row index in tables
TBC = 256                   # table cols (bf16): [h 128 | ssrc 8 | sdst 8 | pad]
SC = 128
NEG = 0.2

bf16 = ml_dtypes.bfloat16


# ----------------------------------------------------------------------------
# host-side schedule construction (pure integer/layout work)
# ----------------------------------------------------------------------------

def _wrap16(a):
    """[L] int -> [128, L//16] int16 (dma_gather/scatter index layout,
    replicated over the 8 q7 partition groups)."""
    w = a.reshape(-1, 16).T.astype(np.int16)
    return np.tile(w, (8, 1))


def _sorted_percore(src, dst):
    percore = []
    for k in range(NCORES):
        sel = (dst >= k * ND) & (dst < (k + 1) * ND)
        s = src[sel].astype(np.int64)
        d = (dst[sel] - k * ND).astype(np.int64)
        deg = np.bincount(d, minlength=ND)
        order = np.argsort(-deg, kind="stable")
        o = np.argsort(d, kind="stable")
        s_sorted = s[o]
        starts = np.zeros(ND + 1, np.int64)
        np.cumsum(deg, out=starts[1:])
        percore.append((deg, order, s_sorted, starts))
    return percore


def _ell(percore, GD):
    """ELL schedule for one edge type at group width GD.

    Returns Rg [NG], cbase [NG], TCpad, gidx [NCORES, ncalls, 128, NCH*8],
    sdti [NCORES, 128, NG*8]."""
    rpc = 128 // GD
    NG = NSLOT // GD
    Rg = np.zeros(NG, np.int64)
    for g in range(NG):
        mx = 1
        lo, hi = g * GD, min((g + 1) * GD, ND)
        for (deg, order, _, _) in percore:
            if lo < hi:
                mx = max(mx, int(deg[order[lo:hi]].max()))
        Rg[g] = ((mx + rpc - 1) // rpc) * rpc
    Cg = Rg // rpc
    cbase = np.zeros(NG, np.int64)
    np.cumsum(Cg[:-1], out=cbase[1:])
    TC = int(Cg.sum())
    TCpad = ((TC + NCH - 1) // NCH) * NCH

    gidx_all, sdti_all = [], []
    for k in range(NCORES):
        deg, order, s_sorted, starts = percore[k]
        gidx = np.full(TCpad * 128, POISON, np.int64)
        sdti = np.full(NG * 128, POISON, np.int64)
        for g in range(NG):
            base = cbase[g]
            for p in range(128):
                rank = g * GD + p % GD
                if rank < ND:
                    sdti[g * 128 + p] = k * ND + int(order[rank])
            for j in range(GD):
                rank = g * GD + j
                if rank < ND:
                    dd = int(order[rank])
                    dg = int(deg[dd])
                    if dg:
                        r = np.arange(dg)
                        pos = (base + r // rpc) * 128 + (r % rpc) * GD + j
                        gidx[pos] = s_sorted[starts[dd]:starts[dd] + dg]
        ncalls = TCpad // NCH
        gidx_all.append(
            _wrap16(gidx).reshape(128, ncalls, NCH * 8).transpose(1, 0, 2))
        sdti_all.append(_wrap16(sdti))
    return dict(Rg=Rg, cbase=cbase, TCpad=TCpad,
                gidx=np.stack(gidx_all), sdti=np.stack(sdti_all))


def _scat(percore):
    out = []
    for k in range(NCORES):
        deg, order, _, _ = percore[k]
        sc = np.zeros(NSLOT, np.int64)
        for rank in range(NSLOT):
            sc[rank] = int(order[rank]) if rank < ND else rank
        out.append(_wrap16(sc))
    return np.stack(out)


def _host_prep(inputs):
    """Layout transforms of the inputs + schedules. No float math beyond
    dtype casts."""
    x = np.asarray(inputs["x"], np.float32)
    W1 = np.asarray(inputs["W1"], np.float32)
    a1s = np.asarray(inputs["a1_src"], np.float32)
    a1d = np.asarray(inputs["a1_dst"], np.float32)
    W2 = np.asarray(inputs["W2"], np.float32)
    a2s = np.asarray(inputs["a2_src"], np.float32)
    a2d = np.asarray(inputs["a2_dst"], np.float32)
    Wc1 = np.asarray(inputs["Wc1"], np.float32)
    Wc2 = np.asarray(inputs["Wc2"], np.float32)

    xT = np.ascontiguousarray(x.T.reshape(2, 128, N)).astype(bf16)

    shared = {}
    shared["w1"] = np.ascontiguousarray(W1.reshape(2, 2, 128, 128)).astype(bf16)
    shared["w1t"] = np.ascontiguousarray(W1.transpose(0, 2, 1)).astype(bf16)
    A1s = np.zeros((2, 128, 8), np.float32)
    A1d = np.zeros((2, 128, 8), np.float32)
    for t in range(2):
        for h in range(8):
            A1s[t, h * 16:(h + 1) * 16, h] = a1s[t, h]
            A1d[t, h * 16:(h + 1) * 16, h] = a1d[t, h]
    shared["a1m"] = np.stack([A1s, A1d], 1).astype(bf16)          # [2, 2, 128, 8]
    shared["w2"] = W2.astype(bf16)                                 # [2, 128, 512]
    shared["w2t"] = np.ascontiguousarray(
        W2.transpose(0, 2, 1).reshape(2, 4, 128, 128)).astype(bf16)
    A2s = np.zeros((2, 512, 8), np.float32)
    A2d = np.zeros((2, 512, 8), np.float32)
    for t in range(2):
        for h in range(8):
            A2s[t, h * 64:(h + 1) * 64, h] = a2s[t, h]
            A2d[t, h * 64:(h + 1) * 64, h] = a2d[t, h]
    shared["a2m"] = np.stack([A2s, A2d], 1).reshape(2, 2, 4, 128, 8).astype(bf16)
    shared["wc1"] = Wc1.astype(bf16)
    shared["wc2"] = Wc2.astype(bf16)

    S1 = np.zeros((128, GD1), np.float32)
    for e in range(128):
        S1[e, e % GD1] = 1.0
    shared["sconst1"] = S1.astype(bf16)
    S2 = np.zeros((128, GD2), np.float32)
    for e in range(128):
        S2[e, e % GD2] = 1.0
    shared["sconst2"] = S2.astype(bf16)
    ident = np.eye(128, dtype=np.float32)
    shared["ident"] = ident.astype(bf16)
    p1 = np.zeros((1, TBC), np.float32)
    p1[0, 128:136] = -600.0
    shared["poison"] = p1.astype(bf16)

    ei_a = np.asarray(inputs["edge_index_a"])
    ei_b = np.asarray(inputs["edge_index_b"])
    pc = {"a": _sorted_percore(ei_a[0], ei_a[1]),
          "b": _sorted_percore(ei_b[0], ei_b[1])}
    sch1 = {t: _ell(pc[t], GD1) for t in "ab"}
    sch2 = {t: _ell(pc[t], GD2) for t in "ab"}
    scat = {t: _scat(pc[t]) for t in "ab"}

    per_core = []
    for k in range(NCORES):
        m = dict(shared)
        m["xTs"] = np.ascontiguousarray(xT[:, :, k * ND:(k + 1) * ND])
        for t in "ab":
            m[f"gidx1_{t}"] = sch1[t]["gidx"][k]
            m[f"gidx2_{t}"] = sch2[t]["gidx"][k]
            m[f"sdti1_{t}"] = sch1[t]["sdti"][k]
            m[f"sdti2_{t}"] = sch2[t]["sdti"][k]
            m[f"scat_{t}"] = scat[t][k]
        per_core.append(m)
    return per_core, sch1, sch2


# ----------------------------------------------------------------------------
# device kernel
# ----------------------------------------------------------------------------

def _patch_queue_aware_lanes():
    """Make Tile's SWDGE DMA semaphore-lane assignment queue-aware: queue q
    gets lanes {2q, 2q+1}.  The stock round-robin mixes queues onto one lane,
    which violates the one-queue-per-semaphore rule of the SWDGE ucode."""
    import concourse.tile_sem_assignment as tsa
    import concourse.mybir as mybir
    if getattr(tsa, "_qaware_patched", False):
        return
    orig = tsa.TileClockTick._assign_tick

    def patched(self, inst):
        if (isinstance(inst, tsa.DMAInst)
                and inst.engine == mybir.EngineType.Pool
                and not isinstance(inst, tsa.bass_isa.UserSyncedRemoteDMADescs)):
            q = getattr(inst, "queue_num", 0) or 0
            tog = getattr(self, "_q_toggle", None)
            if tog is None:
                tog = self._q_toggle = {}
            self.next_sw_dma_idx = (q * 2 + tog.get(q, 0)) % self.swdge_sem_count
            tog[q] = 1 - tog.get(q, 0)
        return orig(self, inst)

    tsa.TileClockTick._assign_tick = patched
    tsa._qaware_patched = True


def _build_nc(sch1, sch2):
    import concourse.bacc as bacc
    import concourse.mybir as mybir
    import concourse.tile as tile

    _patch_queue_aware_lanes()

    BF = mybir.dt.bfloat16
    F32 = mybir.dt.float32
    I16 = mybir.dt.int16
    U8 = mybir.dt.uint8
    AF = mybir.ActivationFunctionType
    OP = mybir.AluOpType
    AX = mybir.AxisListType

    nc = bacc.Bacc("TRN2", target_bir_lowering=False, debug=False,
                   num_devices=NCORES, num_swdge_queues=4)

    def din(name, shape, dt=BF):
        return nc.dram_tensor(name, shape, dt, kind="ExternalInput").ap()

    NSC = NSLOT // 16

    xTs = din("xTs", [2, 128, ND])
    w1 = din("w1", [2, 2, 128, 128])
    w1t = din("w1t", [2, 128, 256])
    a1m = din("a1m", [2, 2, 128, 8])
    w2 = din("w2", [2, 128, 512])
    w2t = din("w2t", [2, 4, 128, 128])
    a2m = din("a2m", [2, 2, 4, 128, 8])
    wc1 = din("wc1", [64, 32])
    wc2 = din("wc2", [32, 2])
    sconst1 = din("sconst1", [128, GD1])
    sconst2 = din("sconst2", [128, GD2])
    ident = din("ident", [128, 128])
    poison = din("poison", [1, TBC])
    gidx1_d = {t: din(f"gidx1_{t}", [sch1[t]["TCpad"] // NCH, 128, NCH * 8], I16)
               for t in "ab"}
    gidx2_d = {t: din(f"gidx2_{t}", [sch2[t]["TCpad"] // NCH, 128, NCH * 8], I16)
               for t in "ab"}
    sdti1_d = {t: din(f"sdti1_{t}", [128, NG1 * 8], I16) for t in "ab"}
    sdti2_d = {t: din(f"sdti2_{t}", [128, NG2 * 8], I16) for t in "ab"}
    scat_d = {t: din(f"scat_{t}", [128, NSC], I16) for t in "ab"}
    out = nc.dram_tensor("out", [ND, 2], F32, kind="ExternalOutput").ap()

    RG = [list(range(NCORES))]
    NBLK = (ND + 127) // 128      # 20 row blocks of <=128 nodes

    with tile.TileContext(nc) as tc:
        with tc.tile_pool(name="dram", bufs=1, space="DRAM") as dpool, \
             tc.tile_pool(name="const", bufs=1) as cpool:

            table1 = {t: dpool.tile([N + 1, TBC], BF, tag=f"tb1{t}",
                                    name=f"table1{t}") for t in "ab"}
            table2 = {t: dpool.tile([N + 1, TBC], BF, tag=f"tb2{t}",
                                    name=f"table2{t}") for t in "ab"}
            slice1 = {t: dpool.tile([ND, TBC], BF, tag=f"sl1{t}",
                                    name=f"slice1{t}") for t in "ab"}
            slice2 = {t: dpool.tile([ND, TBC], BF, tag=f"sl2{t}",
                                    name=f"slice2{t}") for t in "ab"}
            acc1 = dpool.tile([NSLOT, 128], F32, tag="acc1")
            acc2 = dpool.tile([NSLOT, 64], F32, tag="acc2")

            # ---- constants ----
            sc1_sb = cpool.tile([128, GD1], BF)
            nc.sync.dma_start(sc1_sb[:], sconst1[:])
            sc2_sb = cpool.tile([128, GD2], BF)
            nc.sync.dma_start(sc2_sb[:], sconst2[:])
            id_sb = cpool.tile([128, 128], BF)
            nc.sync.dma_start(id_sb[:], ident[:])
            wc1_sb = cpool.tile([64, 32], BF)
            nc.sync.dma_start(wc1_sb[:], wc1[:])
            wc2_sb = cpool.tile([32, 2], BF)
            nc.sync.dma_start(wc2_sb[:], wc2[:])
            scat_sb = {}
            sdti1_sb = {}
            sdti2_sb = {}
            for t in "ab":
                scat_sb[t] = cpool.tile([128, NSC], I16, tag=f"scat{t}",
                                        name=f"scatsb{t}")
                nc.sync.dma_start(scat_sb[t][:], scat_d[t][:])
                sdti1_sb[t] = cpool.tile([128, NG1 * 8], I16, tag=f"sdt1i{t}",
                                         name=f"sdti1sb{t}")
                nc.sync.dma_start(sdti1_sb[t][:], sdti1_d[t][:])
                sdti2_sb[t] = cpool.tile([128, NG2 * 8], I16, tag=f"sdt2i{t}",
                                         name=f"sdti2sb{t}")
                nc.sync.dma_start(sdti2_sb[t][:], sdti2_d[t][:])

            for t in "ab":
                nc.sync.dma_start(table1[t][N:N + 1, :], poison[:])
                nc.sync.dma_start(table2[t][N:N + 1, :], poison[:])

            # zero accumulators
            with tc.tile_pool(name="zacc", bufs=1) as zaccp:
                zt = zaccp.tile([128, NSLOT // 128, 128], F32)
                nc.vector.memset(zt[:], 0.0)
                nc.sync.dma_start(acc1.rearrange("(a p) c -> p a c", p=128), zt[:])
                nc.sync.dma_start(acc2.rearrange("(a p) c -> p a c", p=128),
                                  zt[:, :, 0:64])

            # ---- weight augmentation ----
            w1aug = {}
            w2aug = {}
            with tc.tile_pool(name="aug", bufs=2) as augp, \
                 tc.tile_pool(name="augps", bufs=2, space="PSUM") as augps:
                for ti, t in enumerate("ab"):
                    wa = [cpool.tile([128, 144], BF, tag=f"w1aug{t}{c}",
                                     name=f"w1aug{t}{c}") for c in range(2)]
                    for c in range(2):
                        nc.sync.dma_start(wa[c][:, 0:128], w1[ti, c])
                    for si in range(2):
                        a_sb = augp.tile([128, 8], BF, tag="a1sb")
                        nc.sync.dma_start(a_sb[:], a1m[ti, si])
                        w1t_sb = augp.tile([128, 256], BF, tag="w1tsb")
                        nc.sync.dma_start(w1t_sb[:], w1t[ti])
                        ps = augps.tile([8, 256], F32, tag="wsps")
                        nc.tensor.matmul(out=ps[:], lhsT=a_sb[:], rhs=w1t_sb[:],
                                         start=True, stop=True)
                        s8 = augp.tile([8, 256], BF, tag="ws8")
                        nc.vector.tensor_copy(out=s8[:], in_=ps[:])
                        for c in range(2):
                            tp = augps.tile([128, 8], BF, tag="wstp")
                            nc.tensor.transpose(out=tp[:], in_=s8[:, c * 128:(c + 1) * 128],
                                                identity=id_sb[0:8, 0:8])
                            nc.vector.tensor_copy(
                                out=wa[c][:, 128 + si * 8:136 + si * 8], in_=tp[:])
                    w1aug[t] = wa

                    w2a = cpool.tile([128, 528], BF, tag=f"w2aug{t}", name=f"w2aug{t}")
                    nc.sync.dma_start(w2a[:, 0:512], w2[ti])
                    for si in range(2):
                        ps = augps.tile([8, 128], F32, tag="w2ps")
                        for c in range(4):
                            a_sb = augp.tile([128, 8], BF, tag="a2sb")
                            nc.sync.dma_start(a_sb[:], a2m[ti, si, c])
                            w2t_sb = augp.tile([128, 128], BF, tag="w2tsb")
                            nc.sync.dma_start(w2t_sb[:], w2t[ti, c])
                            nc.tensor.matmul(out=ps[:], lhsT=a_sb[:], rhs=w2t_sb[:],
                                             start=(c == 0), stop=(c == 3))
                        s8 = augp.tile([8, 128], BF, tag="w2s8")
                        nc.vector.tensor_copy(out=s8[:], in_=ps[:])
                        tp = augps.tile([128, 8], BF, tag="w2tp")
                        nc.tensor.transpose(out=tp[:], in_=s8[:],
                                            identity=id_sb[0:8, 0:8])
                        nc.vector.tensor_copy(out=w2a[:, 512 + si * 8:520 + si * 8],
                                              in_=tp[:])
                    w2aug[t] = w2a

            # ---- phase 1: layer-1 table slice (my 2500 rows), then gather ----
            with tc.tile_pool(name="ph1", bufs=4) as p1p, \
                 tc.tile_pool(name="ph1ps", bufs=4, space="PSUM") as p1ps:
                xts = [p1p.tile([128, ND], BF, tag=f"xts{c}", name=f"xtssb{c}",
                                bufs=1) for c in range(2)]
                for c in range(2):
                    nc.sync.dma_start(xts[c][:], xTs[c])
                for ti, t in enumerate("ab"):
                    for i in range(NBLK):
                        lo = i * 128
                        m = min(128, ND - lo)
                        ps = p1ps.tile([128, 144], F32, tag="t1ps")
                        nc.tensor.matmul(out=ps[:m], lhsT=xts[0][:, lo:lo + m],
                                         rhs=w1aug[t][0][:], start=True, stop=False)
                        nc.tensor.matmul(out=ps[:m], lhsT=xts[1][:, lo:lo + m],
                                         rhs=w1aug[t][1][:], start=False, stop=True)
                        o = p1p.tile([128, 144], BF, tag="t1o")
                        if ti == 0:
                            nc.scalar.copy(out=o[:m], in_=ps[:m])
                        else:
                            nc.vector.tensor_copy(out=o[:m], in_=ps[:m])
                        nc.sync.dma_start(slice1[t][lo:lo + m, 0:144], o[:m])
                    nc.gpsimd.collective_compute(
                        "AllGather", mybir.AluOpType.bypass,
                        replica_groups=RG,
                        ins=[slice1[t][:, :].opt()],
                        outs=[table1[t][0:N, :].rearrange(
                            "(k r) c -> k r c", k=NCORES).opt()])

            # ================= layer-1 edge phase (GD1=64) =================
            with tc.tile_pool(name="eg1", bufs=3) as gp, \
                 tc.tile_pool(name="ew1", bufs=3) as wp, \
                 tc.tile_pool(name="es1", bufs=4) as sp, \
                 tc.tile_pool(name="ef1", bufs=1) as fp, \
                 tc.tile_pool(name="park1", bufs=1) as parkp, \
                 tc.tile_pool(name="eps1", bufs=3, space="PSUM") as pp:

                gidx_sb = {}
                sdt = {}
                for t in "ab":
                    ncalls = sch1[t]["TCpad"] // NCH
                    gidx_sb[t] = fp.tile([128, ncalls, NCH * 8], I16,
                                         tag=f"g1x{t}", name=f"gidx1sb{t}")
                    nc.sync.dma_start(gidx_sb[t][:],
                                      gidx1_d[t].rearrange("c p s -> p c s"))
                    sdt[t] = fp.tile([128, NG1, 128], BF, tag=f"sdt1{t}",
                                     name=f"sdt1{t}")

                def sdt1_window(t, g):
                    """Slot-table gather for groups [g, g+NCH)."""
                    sdone = g * 128
                    n = min(NCH * 128, NG1 * 128 - sdone)
                    nc.gpsimd.dma_gather(
                        sdt[t][:, sdone // 128:(sdone + n) // 128, :],
                        table1[t][:, SC:SC + 128],
                        sdti1_sb[t][:, sdone // 16:(sdone + n) // 16],
                        n, n, 128, elem_step=TBC, queue_num=3)

                parks = {t: parkp.tile([128, NBLK, 128], F32, tag=f"park{t}",
                                       name=f"park1{t}") for t in "ab"}
                st = {t: dict(call=-1, G=None, pa=None) for t in "ab"}
                qctr = [0]
                NV = 128 // GD1   # 2 groups per 128-slot batch

                def scatter_acc(t, parks_t, acc, cols, mi):
                    """Fire pending scatter-adds once batches [c0, mi) done."""
                    cuts = {8: (0, 1024), 16: (8, 1024), NBLK: (16, 512)}
                    if mi in cuts:
                        c0, nI = cuts[mi]
                        nc.gpsimd.dma_scatter_add(
                            acc[:], parks_t[:, c0:c0 + nI // 128, :],
                            scat_sb[t][:, c0 * 8:c0 * 8 + nI // 16],
                            nI, nI, cols, queue_num=3)

                def do_group1(t, g):
                    sched = sch1[t]
                    cg = int(sched["Rg"][g] // (128 // GD1))
                    base = int(sched["cbase"][g])
                    s_ = st[t]
                    if g % NV == 0:
                        s_["pa"] = pp.tile([128, 136], F32, tag=f"pa{t}",
                                           name=f"pa1{t}")
                    pa = s_["pa"]
                    row0 = GD1 * (g % NV)
                    done = 0
                    while done < cg:
                        seg = min(NCH - (base + done) % NCH, cg - done)
                        call = (base + done) // NCH
                        coff = (base + done) % NCH
                        if call != s_["call"]:
                            G = gp.tile([128, NCH, TBC], BF, tag=f"G{t}",
                                        name=f"G1{t}")
                            nc.gpsimd.dma_gather(
                                G[:, :, :], table1[t][:],
                                gidx_sb[t][:, call, :],
                                NCH * 128, NCH * 128, TBC,
                                queue_num=qctr[0] % 3)
                            qctr[0] += 1
                            s_["call"] = call
                            s_["G"] = G
                        G = s_["G"]
                        sl = slice(coff, coff + seg)
                        u = sp.tile([128, NCH, 8], F32, tag=f"u{t}",
                                    name=f"u1{t}")
                        nc.vector.tensor_tensor(
                            out=u[:, :seg, :], in0=G[:, sl, SC:SC + 8],
                            in1=sdt[t][:, g, 8:16][:, None, :].to_broadcast(
                                [128, seg, 8]),
                            op=OP.add)
                        phi = sp.tile([128, NCH, 8], F32, tag=f"phi{t}",
                                      name=f"phi1{t}")
                        nc.vector.scalar_tensor_tensor(
                            out=phi[:, :seg, :], in0=u[:, :seg, :], scalar=NEG,
                            in1=u[:, :seg, :], op0=OP.mult, op1=OP.max)
                        q = sp.tile([128, NCH, 8], BF, tag=f"q{t}",
                                    name=f"q1{t}")
                        nc.scalar.activation(out=q[:, :seg, :],
                                             in_=phi[:, :seg, :], func=AF.Exp)
                        W = wp.tile([128, NCH, 136], BF, tag=f"W{t}",
                                    name=f"W1{t}")
                        nc.vector.tensor_tensor(
                            out=W[:, :seg, 0:128].rearrange(
                                "p s (h c) -> p s h c", h=8),
                            in0=G[:, sl, 0:128].rearrange(
                                "p s (h c) -> p s h c", h=8),
                            in1=q[:, :seg, :, None].to_broadcast(
                                [128, seg, 8, 16]),
                            op=OP.mult)
                        nc.scalar.copy(out=W[:, :seg, 128:136], in_=q[:, :seg, :])
                        for s in range(seg):
                            cc = done + s
                            nc.tensor.matmul(
                                out=pa[row0:row0 + GD1, :],
                                lhsT=sc1_sb[:], rhs=W[:, s, 0:136],
                                start=(cc == 0), stop=(cc == cg - 1),
                                skip_group_check=True)
                        done += seg
                    if g % NV == NV - 1:
                        mi = (g * GD1) // 128
                        z8 = sp.tile([128, 8], F32, tag=f"z8{t}",
                                     name=f"z81{t}")
                        nc.vector.tensor_scalar(
                            out=z8[:], in0=pa[:, 128:136], scalar1=1.0,
                            scalar2=1e-30, op0=OP.mult, op1=OP.max)
                        rz = sp.tile([128, 8], F32, tag=f"rz{t}",
                                     name=f"rz1{t}")
                        nc.vector.reciprocal(out=rz[:], in_=z8[:])
                        nc.vector.tensor_tensor(
                            out=parks[t][:, mi, :].rearrange(
                                "p (h c) -> p h c", h=8),
                            in0=pa[:, 0:128].rearrange(
                                "p (h c) -> p h c", h=8),
                            in1=rz[:].to_broadcast([128, 8, 16]),
                            op=OP.mult)
                        scatter_acc(t, parks[t], acc1, 128, mi + 1)

                for t in "ab":
                    for g in range(NG1):
                        if g % NCH == 0:
                            sdt1_window(t, g)
                        do_group1(t, g)

            # ---- combine acc1 -> h1' -> layer-2 table slice ----
            with tc.tile_pool(name="cmb", bufs=4) as cp, \
                 tc.tile_pool(name="cmbps", bufs=4, space="PSUM") as cps:
                for ti, t in enumerate("ab"):
                    for i in range(NBLK):
                        lo = i * 128
                        m = min(128, ND - lo)
                        a = cp.tile([128, 128], F32, tag="c_a")
                        nc.sync.dma_start(a[:m], acc1[lo:lo + m, :])
                        e = cp.tile([128, 128], F32, tag="c_e")
                        nc.scalar.activation(out=e[:m], in_=a[:m], func=AF.Exp,
                                             scale=0.5)
                        em1 = cp.tile([128, 128], F32, tag="c_em1")
                        nc.vector.tensor_scalar(out=em1[:m], in0=e[:m],
                                                scalar1=-1.0,
                                                scalar2=None, op0=OP.add)
                        xm = cp.tile([128, 128], F32, tag="c_xm")
                        nc.vector.tensor_scalar(out=xm[:m], in0=a[:m],
                                                scalar1=0.5,
                                                scalar2=None, op0=OP.mult)
                        mk = cp.tile([128, 128], mybir.dt.uint8, tag="c_mk")
                        nc.vector.tensor_scalar(out=mk[:m], in0=a[:m],
                                                scalar1=0.0,
                                                scalar2=None, op0=OP.is_gt)
                        h = cp.tile([128, 128], BF, tag="c_h")
                        nc.vector.select(out=h[:m], mask=mk[:m], on_true=xm[:m],
                                         on_false=em1[:m])
                        # transpose for the s2 projections
                        tps = cps.tile([128, 128], BF, tag="c_tp")
                        nc.tensor.transpose(out=tps[:, :m], in_=h[:m, :],
                                            identity=id_sb[:m, :m])
                        ht = cp.tile([128, 128], BF, tag="c_ht")
                        nc.scalar.copy(out=ht[:, :m], in_=tps[:, :m])
                        ps2 = cps.tile([128, 16], F32, tag="c_s2")
                        nc.tensor.matmul(out=ps2[:m], lhsT=ht[:, :m],
                                         rhs=w2aug[t][:, 512:528],
                                         start=True, stop=True)
                        o2 = cp.tile([128, 144], BF, tag="c_o2")
                        nc.vector.tensor_copy(out=o2[:m, 0:128], in_=h[:m])
                        nc.scalar.copy(out=o2[:m, 128:144], in_=ps2[:m])
                        nc.sync.dma_start(slice2[t][lo:lo + m, 0:144],
                                          o2[:m])
                    nc.gpsimd.collective_compute(
                        "AllGather", mybir.AluOpType.bypass,
                        replica_groups=RG,
                        ins=[slice2[t][:, :].opt()],
                        outs=[table2[t][0:N, :].rearrange(
                            "(k r) c -> k r c", k=NCORES).opt()])

            # ================= layer-2 edge phase (GD2=16) =================
            with tc.tile_pool(name="eg2", bufs=3) as gp2, \
                 tc.tile_pool(name="ew2", bufs=3) as wp2, \
                 tc.tile_pool(name="es2", bufs=4) as sp2, \
                 tc.tile_pool(name="ms2", bufs=2) as msp, \
                 tc.tile_pool(name="ef2", bufs=1) as fp2, \
                 tc.tile_pool(name="park2", bufs=1) as park2p, \
                 tc.tile_pool(name="sst2", bufs=2) as sstp, \
                 tc.tile_pool(name="emt2", bufs=3, space="PSUM") as mtp, \
                 tc.tile_pool(name="ezp2", bufs=1, space="PSUM") as zpp, \
                 tc.tile_pool(name="epp2", bufs=2, space="PSUM") as php:

                gidx2_sb = {}
                sdt2 = {}
                for t in "ab":
                    ncalls = sch2[t]["TCpad"] // NCH
                    gidx2_sb[t] = fp2.tile([128, ncalls, NCH * 8], I16,
                                           tag=f"g2x{t}", name=f"gidx2sb{t}")
                    nc.sync.dma_start(gidx2_sb[t][:],
                                      gidx2_d[t].rearrange("c p s -> p c s"))
                    sdt2[t] = fp2.tile([128, NG2, 8], BF, tag=f"sdt2{t}",
                                       name=f"sdt2{t}")

                def sdt2_window(t, g):
                    """Slot-table gather + s_dst compaction for [g, g+NCH)."""
                    sdone = g * 128
                    n = min(NCH * 128, NG2 * 128 - sdone)
                    stg = sstp.tile([128, NCH, 128], BF, tag=f"sst{t}",
                                    name=f"sst2{t}")
                    nc.gpsimd.dma_gather(
                        stg[:, 0:n // 128, :], table2[t][:, SC:SC + 128],
                        sdti2_sb[t][:, sdone // 16:(sdone + n) // 16],
                        n, n, 128, elem_step=TBC, queue_num=3)
                    nc.vector.tensor_copy(
                        out=sdt2[t][:, sdone // 128:(sdone + n) // 128, :],
                        in_=stg[:, 0:n // 128, 8:16])

                parks2 = {t: park2p.tile([128, NBLK, 64], F32, tag=f"park2{t}",
                                         name=f"park2{t}") for t in "ab"}
                st2 = {t: dict(call=-1, G=None, Q=None, zps=None, msb=None)
                       for t in "ab"}
                qctr2 = [0]
                NV2 = 128 // GD2   # 8 groups per 128-slot batch

                def scatter_acc2(t, mi):
                    cuts = {8: (0, 1024), 16: (8, 1024), NBLK: (16, 512)}
                    if mi in cuts:
                        c0, nI = cuts[mi]
                        nc.gpsimd.dma_scatter_add(
                            acc2[:], parks2[t][:, c0:c0 + nI // 128, :],
                            scat_sb[t][:, c0 * 8:c0 * 8 + nI // 16],
                            nI, nI, 64, queue_num=3)

                def do_group2(t, g):
                    sched = sch2[t]
                    cg = int(sched["Rg"][g] // (128 // GD2))
                    base = int(sched["cbase"][g])
                    s_ = st2[t]
                    if g % NV2 == 0:
                        s_["zps"] = zpp.tile([8, 128], F32, tag=f"zp{t}",
                                             name=f"zps2{t}")
                        # [c1, h, grp*16+d]: per-head slice is 2D-contiguous
                        s_["msb"] = msp.tile([128, 8, 128], BF, tag=f"ms{t}",
                                             name=f"msb2{t}")
                    zps, msb = s_["zps"], s_["msb"]
                    mt = mtp.tile([128, 128], F32, tag="mt", name=f"mt2{t}")
                    zcol = GD2 * (g % NV2)
                    done = 0
                    while done < cg:
                        seg = min(NCH - (base + done) % NCH, cg - done)
                        call = (base + done) // NCH
                        coff = (base + done) % NCH
                        if call != s_["call"]:
                            G = gp2.tile([128, NCH, TBC], BF, tag=f"G{t}",
                                         name=f"G2{t}")
                            nc.gpsimd.dma_gather(
                                G[:, :, :], table2[t][:],
                                gidx2_sb[t][:, call, :],
                                NCH * 128, NCH * 128, TBC,
                                queue_num=qctr2[0] % 3)
                            qctr2[0] += 1
                            s_["call"] = call
                            s_["G"] = G
                        G = s_["G"]
                        sl = slice(coff, coff + seg)
                        u = sp2.tile([128, NCH, 8], F32, tag=f"u{t}",
                                     name=f"u2{t}")
                        nc.vector.tensor_tensor(
                            out=u[:, :seg, :], in0=G[:, sl, SC:SC + 8],
                            in1=sdt2[t][:, g, :][:, None, :].to_broadcast(
                                [128, seg, 8]),
                            op=OP.add)
                        phi = sp2.tile([128, NCH, 8], F32, tag=f"phi{t}",
                                       name=f"phi2{t}")
                        nc.vector.scalar_tensor_tensor(
                            out=phi[:, :seg, :], in0=u[:, :seg, :], scalar=NEG,
                            in1=u[:, :seg, :], op0=OP.mult, op1=OP.max)
                        q = sp2.tile([128, NCH, 8], BF, tag=f"q{t}",
                                     name=f"q2{t}")
                        nc.scalar.activation(out=q[:, :seg, :],
                                             in_=phi[:, :seg, :], func=AF.Exp)
                        Q = wp2.tile([128, NCH, 128], BF, tag=f"Q{t}",
                                     name=f"Q2{t}")
                        nc.vector.tensor_tensor(
                            out=Q[:, :seg, :].rearrange(
                                "p s (h d) -> p s h d", h=8),
                            in0=q[:, :seg, :, None].to_broadcast(
                                [128, seg, 8, GD2]),
                            in1=sc2_sb[:, None, None, :].to_broadcast(
                                [128, seg, 8, GD2]),
                            op=OP.mult)
                        for s in range(seg):
                            cc = done + s
                            first = cc == 0
                            last = cc == cg - 1
                            nc.tensor.matmul(
                                out=mt[:], lhsT=G[:, coff + s, 0:128],
                                rhs=Q[:, s, :],
                                start=first, stop=last,
                                skip_group_check=True)
                            nc.tensor.matmul(
                                out=zps[:, zcol:zcol + GD2],
                                lhsT=q[:, s, :], rhs=sc2_sb[:],
                                start=first, stop=last,
                                skip_group_check=True)
                        done += seg
                    # park M_T for this group (cast f32 psum -> bf16 sbuf)
                    gb = (g % NV2) * GD2
                    nc.vector.tensor_copy(
                        out=msb[:, :, gb:gb + GD2],
                        in_=mt[:].rearrange("p (h d) -> p h d", h=8))
                    if g % NV2 == NV2 - 1:
                        mi = (g * GD2) // 128
                        zsb = sp2.tile([8, 128], BF, tag=f"zs{t}",
                                       name=f"zsb2{t}")
                        nc.vector.tensor_copy(out=zsb[:], in_=zps[:])
                        ztp = zpp.tile([128, 8], BF, tag="zt",
                                       name=f"ztp2{t}")
                        nc.tensor.transpose(out=ztp[:], in_=zsb[:],
                                            identity=id_sb[0:8, 0:8])
                        z8 = sp2.tile([128, 8], F32, tag=f"z8{t}",
                                      name=f"z82{t}")
                        nc.vector.tensor_scalar(
                            out=z8[:], in0=ztp[:], scalar1=8.0,
                            scalar2=1e-30, op0=OP.mult, op1=OP.max)
                        rz = sp2.tile([128, 8], F32, tag=f"rz{t}",
                                      name=f"rz2{t}")
                        nc.vector.reciprocal(out=rz[:], in_=z8[:])
                        pv = php.tile([128, 8, 64], F32, tag="pv",
                                      name=f"pv2{t}")
                        for h in range(8):
                            nc.tensor.matmul(
                                out=pv[:, h, :],
                                lhsT=msb[:, h, :],
                                rhs=w2aug[t][:, h * 64:(h + 1) * 64],
                                start=True, stop=True,
                                skip_group_check=True)
                        tmp = sp2.tile([128, 8, 64], F32, tag=f"tmp{t}",
                                       name=f"tmp2{t}")
                        nc.vector.tensor_tensor(
                            out=tmp[:], in0=pv[:],
                            in1=rz[:, :, None].to_broadcast([128, 8, 64]),
                            op=OP.mult)
                        nc.vector.tensor_reduce(
                            out=parks2[t][:, mi, :, None],
                            in_=tmp[:].rearrange("p h c -> p c h"),
                            axis=AX.X, op=OP.add)
                        scatter_acc2(t, mi + 1)

                for t in "ab":
                    for g in range(NG2):
                        if g % NCH == 0:
                            sdt2_window(t, g)
                        do_group2(t, g)

            # ---- classifier ----
            with tc.tile_pool(name="cls", bufs=4) as clsp, \
                 tc.tile_pool(name="clsps", bufs=2, space="PSUM") as clsps:
                for i in range(NBLK):
                    lo = i * 128
                    m = min(128, ND - lo)
                    a = clsp.tile([128, 64], F32, tag="k_a")
                    nc.sync.dma_start(a[:m], acc2[lo:lo + m, :])
                    e = clsp.tile([128, 64], F32, tag="k_e")
                    nc.scalar.activation(out=e[:m], in_=a[:m], func=AF.Exp,
                                         scale=0.5)
                    em1 = clsp.tile([128, 64], F32, tag="k_em1")
                    nc.vector.tensor_scalar(out=em1[:m], in0=e[:m], scalar1=-1.0,
                                            scalar2=None, op0=OP.add)
                    xm = clsp.tile([128, 64], F32, tag="k_xm")
                    nc.vector.tensor_scalar(out=xm[:m], in0=a[:m], scalar1=0.5,
                                            scalar2=None, op0=OP.mult)
                    mk = clsp.tile([128, 64], mybir.dt.uint8, tag="k_mk")
                    nc.vector.tensor_scalar(out=mk[:m], in0=a[:m], scalar1=0.0,
                                            scalar2=None, op0=OP.is_gt)
                    h = clsp.tile([128, 64], BF, tag="k_h")
                    nc.vector.select(out=h[:m], mask=mk[:m], on_true=xm[:m],
                                     on_false=em1[:m])
                    tps = clsps.tile([64, 128], BF, tag="k_t1")
                    nc.tensor.transpose(out=tps[:, :m], in_=h[:m, :],
                                        identity=id_sb[:m, :m])
                    h3t = clsp.tile([64, 128], BF, tag="k_h3t")
                    nc.scalar.copy(out=h3t[:, :m], in_=tps[:, :m])
                    z1 = clsps.tile([128, 32], F32, tag="k_z1")
                    nc.tensor.matmul(out=z1[:m], lhsT=h3t[:, :m], rhs=wc1_sb[:],
                                     start=True, stop=True)
                    z1s = clsp.tile([128, 32], BF, tag="k_z1s")
                    nc.scalar.activation(out=z1s[:m], in_=z1[:m], func=AF.Relu)
                    t2ps = clsps.tile([32, 128], BF, tag="k_t2")
                    nc.tensor.transpose(out=t2ps[:, :m], in_=z1s[:m, :],
                                        identity=id_sb[:m, :m])
                    z1t = clsp.tile([32, 128], BF, tag="k_z1t")
                    nc.scalar.copy(out=z1t[:, :m], in_=t2ps[:, :m])
                    lg = clsps.tile([128, 2], F32, tag="k_lg")
                    nc.tensor.matmul(out=lg[:m], lhsT=z1t[:, :m], rhs=wc2_sb[:],
                                     start=True, stop=True)
                    lo_ = clsp.tile([128, 2], F32, tag="k_out")
                    nc.vector.tensor_copy(out=lo_[:m], in_=lg[:m])
                    nc.sync.dma_start(out[lo:lo + m, :], lo_[:m])

    nc.compile()
    return nc


# ----------------------------------------------------------------------------
# entry point
# ----------------------------------------------------------------------------

_CACHE = {}


def _prepare(inputs):
    per_core, sch1, sch2 = _host_prep(inputs)
    key = (tuple(sch1[t]["TCpad"] for t in "ab"),
           tuple(sch2[t]["TCpad"] for t in "ab"),
           tuple(tuple(sch1[t]["Rg"]) for t in "ab"),
           tuple(tuple(sch2[t]["Rg"]) for t in "ab"))
    if key not in _CACHE:
        _CACHE.clear()
        _CACHE[key] = _build_nc(sch1, sch2)
    return _CACHE[key], per_core


def _run(nc, per_core, **kw):
    from concourse import bass_utils
    return bass_utils.run_bass_kernel_spmd(nc, per_core,
                                           core_ids=list(range(NCORES)), **kw)


def kernel(**inputs):
    nc, per_core = _prepare(inputs)
    res = _run(nc, per_core)
    return np.concatenate([res.results[k]["out"] for k in range(NCORES)], 0)


# revision 28
# speedup vs baseline: 1.2315x; 1.2315x over previous
"""Trainium2 Bass kernel for nn_BotAwareGAT (2-layer hetero GAT + MLP).

Strategy (8 NeuronCores, SPMD, dst-partitioned):
  - Core k owns dst nodes [k*2500, (k+1)*2500).
  - Node tables (one per edge type per layer) hold 512B rows:
    [h(128 bf16) | s_src(8 bf16) | s_dst(8 bf16) | pad].  Table builds are
    SHARDED: each core computes its 2500 rows, then AllGather fills the
    replicated table.  Layer-2 tables hold h1' (post-ELU layer-1 output),
    NOT h2: since out2 = W2^T (sum_e alpha_e h1'[src_e]) the W2 projection
    is linear and is applied per-dst AFTER aggregation, so edges only ever
    gather 512B rows (vs 1280B for h2-space rows).
  - Per-edge softmax-aggregation uses an ELL schedule (degree-sorted dst
    groups); messages fetched with batched dma_gather (2048 idx/call).
    L1 (GD=64): psum += S^T @ (q (*) h) per 128-edge chunk (S one-hot).
    L2 (GD=16): M_T[c1, h*16+d] += G_h1^T @ (q (*) S) per chunk; per
    128-dst batch M_T is normalized and projected through W2 per head.
  - Padding edges point at a poison row (s_src=-600 => q underflows to 0).
"""

import numpy as np
import ml_dtypes

N = 20000
NCORES = 8
ND = N // NCORES            # 2500 dst nodes per core
NSLOT = 2560                # padded slots per core
GD1 = 64                    # L1 dsts per group
GD2 = 16                    # L2 dsts per group
NG1 = NSLOT // GD1          # 40
NG2 = NSLOT // GD2          # 160
NCH = 8                     # chunks per dma_gather call (1024 idx)
POISON = N                  # poison row index in tables
TBC = 256                   # table cols (bf16): [h 128 | ssrc 8 | sdst 8 | pad]
SC = 128
NEG = 0.2

bf16 = ml_dtypes.bfloat16


# ----------------------------------------------------------------------------
# host-side schedule construction (pure integer/layout work)
# ----------------------------------------------------------------------------

def _wrap16(a):
    """[L] int -> [128, L//16] int16 (dma_gather/scatter index layout,
    replicated over the 8 q7 partition groups)."""
    w = a.reshape(-1, 16).T.astype(np.int16)
    return np.tile(w, (8, 1))


def _sorted_percore(src, dst):
    percore = []
    for k in range(NCORES):
        sel = (dst >= k * ND) & (dst < (k + 1) * ND)
        s = src[sel].astype(np.int64)
        d = (dst[sel] - k * ND).astype(np.int64)
        deg = np.bincount(d, minlength=ND)
        order = np.argsort(-deg, kind="stable")
        o = np.argsort(d, kind="stable")
        s_sorted = s[o]
        starts = np.zeros(ND + 1, np.int64)
        np.cumsum(deg, out=starts[1:])
        percore.append((deg, order, s_sorted, starts))
    return percore


def _ell(percore, GD):
    """ELL schedule for one edge type at group width GD.

    Returns Rg [NG], cbase [NG], TCpad, gidx [NCORES, ncalls, 128, NCH*8],
    sdti [NCORES, 128, NG*8]."""
    rpc = 128 // GD
    NG = NSLOT // GD
    Rg = np.zeros(NG, np.int64)
    for g in range(NG):
        mx = 1
        lo, hi = g * GD, min((g + 1) * GD, ND)
        for (deg, order, _, _) in percore:
            if lo < hi:
                mx = max(mx, int(deg[order[lo:hi]].max()))
        Rg[g] = ((mx + rpc - 1) // rpc) * rpc
    Cg = Rg // rpc
    cbase = np.zeros(NG, np.int64)
    np.cumsum(Cg[:-1], out=cbase[1:])
    TC = int(Cg.sum())
    TCpad = ((TC + NCH - 1) // NCH) * NCH

    gidx_all, sdti_all = [], []
    for k in range(NCORES):
        deg, order, s_sorted, starts = percore[k]
        gidx = np.full(TCpad * 128, POISON, np.int64)
        sdti = np.full(NG * 128, POISON, np.int64)
        for g in range(NG):
            base = cbase[g]
            for p in range(128):
                rank = g * GD + p % GD
                if rank < ND:
                    sdti[g * 128 + p] = k * ND + int(order[rank])
            for j in range(GD):
                rank = g * GD + j
                if rank < ND:
                    dd = int(order[rank])
                    dg = int(deg[dd])
                    if dg:
                        r = np.arange(dg)
                        pos = (base + r // rpc) * 128 + (r % rpc) * GD + j
                        gidx[pos] = s_sorted[starts[dd]:starts[dd] + dg]
        ncalls = TCpad // NCH
        gidx_all.append(
            _wrap16(gidx).reshape(128, ncalls, NCH * 8).transpose(1, 0, 2))
        sdti_all.append(_wrap16(sdti))
    return dict(Rg=Rg, cbase=cbase, TCpad=TCpad,
                gidx=np.stack(gidx_all), sdti=np.stack(sdti_all))


def _scat(percore):
    out = []
    for k in range(NCORES):
        deg, order, _, _ = percore[k]
        sc = np.zeros(NSLOT, np.int64)
        for rank in range(NSLOT):
            sc[rank] = int(order[rank]) if rank < ND else rank
        out.append(_wrap16(sc))
    return np.stack(out)


def _host_prep(inputs):
    """Layout transforms of the inputs + schedules. No float math beyond
    dtype casts."""
    x = np.asarray(inputs["x"], np.float32)
    W1 = np.asarray(inputs["W1"], np.float32)
    a1s = np.asarray(inputs["a1_src"], np.float32)
    a1d = np.asarray(inputs["a1_dst"], np.float32)
    W2 = np.asarray(inputs["W2"], np.float32)
    a2s = np.asarray(inputs["a2_src"], np.float32)
    a2d = np.asarray(inputs["a2_dst"], np.float32)
    Wc1 = np.asarray(inputs["Wc1"], np.float32)
    Wc2 = np.asarray(inputs["Wc2"], np.float32)

    xT = np.ascontiguousarray(x.T.reshape(2, 128, N)).astype(bf16)

    shared = {}
    shared["w1"] = np.ascontiguousarray(W1.reshape(2, 2, 128, 128)).astype(bf16)
    shared["w1t"] = np.ascontiguousarray(W1.transpose(0, 2, 1)).astype(bf16)
    A1s = np.zeros((2, 128, 8), np.float32)
    A1d = np.zeros((2, 128, 8), np.float32)
    for t in range(2):
        for h in range(8):
            A1s[t, h * 16:(h + 1) * 16, h] = a1s[t, h]
            A1d[t, h * 16:(h + 1) * 16, h] = a1d[t, h]
    shared["a1m"] = np.stack([A1s, A1d], 1).astype(bf16)          # [2, 2, 128, 8]
    shared["w2"] = W2.astype(bf16)                                 # [2, 128, 512]
    shared["w2t"] = np.ascontiguousarray(
        W2.transpose(0, 2, 1).reshape(2, 4, 128, 128)).astype(bf16)
    A2s = np.zeros((2, 512, 8), np.float32)
    A2d = np.zeros((2, 512, 8), np.float32)
    for t in range(2):
        for h in range(8):
            A2s[t, h * 64:(h + 1) * 64, h] = a2s[t, h]
            A2d[t, h * 64:(h + 1) * 64, h] = a2d[t, h]
    shared["a2m"] = np.stack([A2s, A2d], 1).reshape(2, 2, 4, 128, 8).astype(bf16)
    shared["wc1"] = Wc1.astype(bf16)
    shared["wc2"] = Wc2.astype(bf16)

    S1 = np.zeros((128, GD1), np.float32)
    for e in range(128):
        S1[e, e % GD1] = 1.0
    shared["sconst1"] = S1.astype(bf16)
    S2 = np.zeros((128, GD2), np.float32)
    for e in range(128):
        S2[e, e % GD2] = 1.0
    shared["sconst2"] = S2.astype(bf16)
    ident = np.eye(128, dtype=np.float32)
    shared["ident"] = ident.astype(bf16)
    p1 = np.zeros((1, TBC), np.float32)
    p1[0, 128:136] = -600.0
    shared["poison"] = p1.astype(bf16)

    ei_a = np.asarray(inputs["edge_index_a"])
    ei_b = np.asarray(inputs["edge_index_b"])
    pc = {"a": _sorted_percore(ei_a[0], ei_a[1]),
          "b": _sorted_percore(ei_b[0], ei_b[1])}
    sch1 = {t: _ell(pc[t], GD1) for t in "ab"}
    sch2 = {t: _ell(pc[t], GD2) for t in "ab"}
    scat = {t: _scat(pc[t]) for t in "ab"}

    per_core = []
    for k in range(NCORES):
        m = dict(shared)
        m["xTs"] = np.ascontiguousarray(xT[:, :, k * ND:(k + 1) * ND])
        for t in "ab":
            m[f"gidx1_{t}"] = sch1[t]["gidx"][k]
            m[f"gidx2_{t}"] = sch2[t]["gidx"][k]
            m[f"sdti1_{t}"] = sch1[t]["sdti"][k]
            m[f"sdti2_{t}"] = sch2[t]["sdti"][k]
            m[f"scat_{t}"] = scat[t][k]
        per_core.append(m)
    return per_core, sch1, sch2


# ----------------------------------------------------------------------------
# device kernel
# ----------------------------------------------------------------------------

def _patch_queue_aware_lanes():
    """Make Tile's SWDGE DMA semaphore-lane assignment queue-aware: queue q
    gets lanes {2q, 2q+1}.  The stock round-robin mixes queues onto one lane,
    which violates the one-queue-per-semaphore rule of the SWDGE ucode."""
    import concourse.tile_sem_assignment as tsa
    import concourse.mybir as mybir
    if getattr(tsa, "_qaware_patched", False):
        return
    orig = tsa.TileClockTick._assign_tick

    def patched(self, inst):
        if (isinstance(inst, tsa.DMAInst)
                and inst.engine == mybir.EngineType.Pool
                and not isinstance(inst, tsa.bass_isa.UserSyncedRemoteDMADescs)):
            q = getattr(inst, "queue_num", 0) or 0
            tog = getattr(self, "_q_toggle", None)
            if tog is None:
                tog = self._q_toggle = {}
            self.next_sw_dma_idx = (q * 2 + tog.get(q, 0)) % self.swdge_sem_count
            tog[q] = 1 - tog.get(q, 0)
        return orig(self, inst)

    tsa.TileClockTick._assign_tick = patched
    tsa._qaware_patched = True


def _build_nc(sch1, sch2):
    import concourse.bacc as bacc
    import concourse.mybir as mybir
    import concourse.tile as tile

    _patch_queue_aware_lanes()

    BF = mybir.dt.bfloat16
    F32 = mybir.dt.float32
    I16 = mybir.dt.int16
    U8 = mybir.dt.uint8
    AF = mybir.ActivationFunctionType
    OP = mybir.AluOpType
    AX = mybir.AxisListType

    nc = bacc.Bacc("TRN2", target_bir_lowering=False, debug=False,
                   num_devices=NCORES, num_swdge_queues=4)

    def din(name, shape, dt=BF):
        return nc.dram_tensor(name, shape, dt, kind="ExternalInput").ap()

    NSC = NSLOT // 16

    xTs = din("xTs", [2, 128, ND])
    w1 = din("w1", [2, 2, 128, 128])
    w1t = din("w1t", [2, 128, 256])
    a1m = din("a1m", [2, 2, 128, 8])
    w2 = din("w2", [2, 128, 512])
    w2t = din("w2t", [2, 4, 128, 128])
    a2m = din("a2m", [2, 2, 4, 128, 8])
    wc1 = din("wc1", [64, 32])
    wc2 = din("wc2", [32, 2])
    sconst1 = din("sconst1", [128, GD1])
    sconst2 = din("sconst2", [128, GD2])
    ident = din("ident", [128, 128])
    poison = din("poison", [1, TBC])
    gidx1_d = {t: din(f"gidx1_{t}", [sch1[t]["TCpad"] // NCH, 128, NCH * 8], I16)
               for t in "ab"}
    gidx2_d = {t: din(f"gidx2_{t}", [sch2[t]["TCpad"] // NCH, 128, NCH * 8], I16)
               for t in "ab"}
    sdti1_d = {t: din(f"sdti1_{t}", [128, NG1 * 8], I16) for t in "ab"}
    sdti2_d = {t: din(f"sdti2_{t}", [128, NG2 * 8], I16) for t in "ab"}
    scat_d = {t: din(f"scat_{t}", [128, NSC], I16) for t in "ab"}
    out = nc.dram_tensor("out", [ND, 2], F32, kind="ExternalOutput").ap()

    RG = [list(range(NCORES))]
    NBLK = (ND + 127) // 128      # 20 row blocks of <=128 nodes

    with tile.TileContext(nc) as tc:
        with tc.tile_pool(name="dram", bufs=1, space="DRAM") as dpool, \
             tc.tile_pool(name="const", bufs=1) as cpool:

            table1 = {t: dpool.tile([N + 1, TBC], BF, tag=f"tb1{t}",
                                    name=f"table1{t}") for t in "ab"}
            table2 = {t: dpool.tile([N + 1, TBC], BF, tag=f"tb2{t}",
                                    name=f"table2{t}") for t in "ab"}
            slice1 = {t: dpool.tile([ND, TBC], BF, tag=f"sl1{t}",
                                    name=f"slice1{t}") for t in "ab"}
            slice2 = {t: dpool.tile([ND, TBC], BF, tag=f"sl2{t}",
                                    name=f"slice2{t}") for t in "ab"}
            acc1 = dpool.tile([NSLOT, 128], F32, tag="acc1")
            acc2 = dpool.tile([NSLOT, 64], F32, tag="acc2")

            # ---- constants ----
            sc1_sb = cpool.tile([128, GD1], BF)
            nc.sync.dma_start(sc1_sb[:], sconst1[:])
            sc2_sb = cpool.tile([128, GD2], BF)
            nc.sync.dma_start(sc2_sb[:], sconst2[:])
            id_sb = cpool.tile([128, 128], BF)
            nc.sync.dma_start(id_sb[:], ident[:])
            wc1_sb = cpool.tile([64, 32], BF)
            nc.sync.dma_start(wc1_sb[:], wc1[:])
            wc2_sb = cpool.tile([32, 2], BF)
            nc.sync.dma_start(wc2_sb[:], wc2[:])
            scat_sb = {}
            sdti1_sb = {}
            sdti2_sb = {}
            for t in "ab":
                scat_sb[t] = cpool.tile([128, NSC], I16, tag=f"scat{t}",
                                        name=f"scatsb{t}")
                nc.sync.dma_start(scat_sb[t][:], scat_d[t][:])
                sdti1_sb[t] = cpool.tile([128, NG1 * 8], I16, tag=f"sdt1i{t}",
                                         name=f"sdti1sb{t}")
                nc.sync.dma_start(sdti1_sb[t][:], sdti1_d[t][:])
                sdti2_sb[t] = cpool.tile([128, NG2 * 8], I16, tag=f"sdt2i{t}",
                                         name=f"sdti2sb{t}")
                nc.sync.dma_start(sdti2_sb[t][:], sdti2_d[t][:])

            for t in "ab":
                nc.sync.dma_start(table1[t][N:N + 1, :], poison[:])
                nc.sync.dma_start(table2[t][N:N + 1, :], poison[:])

            # zero accumulators
            with tc.tile_pool(name="zacc", bufs=1) as zaccp:
                zt = zaccp.tile([128, NSLOT // 128, 128], F32)
                nc.vector.memset(zt[:], 0.0)
                nc.sync.dma_start(acc1.rearrange("(a p) c -> p a c", p=128), zt[:])
                nc.sync.dma_start(acc2.rearrange("(a p) c -> p a c", p=128),
                                  zt[:, :, 0:64])

            # ---- weight augmentation ----
            w1aug = {}
            w2aug = {}
            with tc.tile_pool(name="aug", bufs=2) as augp, \
                 tc.tile_pool(name="augps", bufs=2, space="PSUM") as augps:
                for ti, t in enumerate("ab"):
                    wa = [cpool.tile([128, 144], BF, tag=f"w1aug{t}{c}",
                                     name=f"w1aug{t}{c}") for c in range(2)]
                    for c in range(2):
                        nc.sync.dma_start(wa[c][:, 0:128], w1[ti, c])
                    for si in range(2):
                        a_sb = augp.tile([128, 8], BF, tag="a1sb")
                        nc.sync.dma_start(a_sb[:], a1m[ti, si])
                        w1t_sb = augp.tile([128, 256], BF, tag="w1tsb")
                        nc.sync.dma_start(w1t_sb[:], w1t[ti])
                        ps = augps.tile([8, 256], F32, tag="wsps")
                        nc.tensor.matmul(out=ps[:], lhsT=a_sb[:], rhs=w1t_sb[:],
                                         start=True, stop=True)
                        s8 = augp.tile([8, 256], BF, tag="ws8")
                        nc.vector.tensor_copy(out=s8[:], in_=ps[:])
                        for c in range(2):
                            tp = augps.tile([128, 8], BF, tag="wstp")
                            nc.tensor.transpose(out=tp[:], in_=s8[:, c * 128:(c + 1) * 128],
                                                identity=id_sb[0:8, 0:8])
                            nc.vector.tensor_copy(
                                out=wa[c][:, 128 + si * 8:136 + si * 8], in_=tp[:])
                    w1aug[t] = wa

                    w2a = cpool.tile([128, 528], BF, tag=f"w2aug{t}", name=f"w2aug{t}")
                    nc.sync.dma_start(w2a[:, 0:512], w2[ti])
                    for si in range(2):
                        ps = augps.tile([8, 128], F32, tag="w2ps")
                        for c in range(4):
                            a_sb = augp.tile([128, 8], BF, tag="a2sb")
                            nc.sync.dma_start(a_sb[:], a2m[ti, si, c])
                            w2t_sb = augp.tile([128, 128], BF, tag="w2tsb")
                            nc.sync.dma_start(w2t_sb[:], w2t[ti, c])
                            nc.tensor.matmul(out=ps[:], lhsT=a_sb[:], rhs=w2t_sb[:],
                                             start=(c == 0), stop=(c == 3))
                        s8 = augp.tile([8, 128], BF, tag="w2s8")
                        nc.vector.tensor_copy(out=s8[:], in_=ps[:])
                        tp = augps.tile([128, 8], BF, tag="w2tp")
                        nc.tensor.transpose(out=tp[:], in_=s8[:],
                                            identity=id_sb[0:8, 0:8])
                        nc.vector.tensor_copy(out=w2a[:, 512 + si * 8:520 + si * 8],
                                              in_=tp[:])
                    w2aug[t] = w2a

            # ---- phase 1: layer-1 table slice (my 2500 rows), then gather ----
            with tc.tile_pool(name="ph1", bufs=4) as p1p, \
                 tc.tile_pool(name="ph1ps", bufs=4, space="PSUM") as p1ps:
                xts = [p1p.tile([128, ND], BF, tag=f"xts{c}", name=f"xtssb{c}",
                                bufs=1) for c in range(2)]
                for c in range(2):
                    nc.sync.dma_start(xts[c][:], xTs[c])
                for i in range(NBLK):
                    lo = i * 128
                    m = min(128, ND - lo)
                    for ti, t in enumerate("ab"):
                        ps = p1ps.tile([128, 144], F32, tag="t1ps")
                        nc.tensor.matmul(out=ps[:m], lhsT=xts[0][:, lo:lo + m],
                                         rhs=w1aug[t][0][:], start=True, stop=False)
                        nc.tensor.matmul(out=ps[:m], lhsT=xts[1][:, lo:lo + m],
                                         rhs=w1aug[t][1][:], start=False, stop=True)
                        o = p1p.tile([128, 144], BF, tag="t1o")
                        if ti == 0:
                            nc.scalar.copy(out=o[:m], in_=ps[:m])
                        else:
                            nc.vector.tensor_copy(out=o[:m], in_=ps[:m])
                        nc.sync.dma_start(slice1[t][lo:lo + m, 0:144], o[:m])
                for t in "ab":
                    nc.gpsimd.collective_compute(
                        "AllGather", mybir.AluOpType.bypass,
                        replica_groups=RG,
                        ins=[slice1[t][:, :].opt()],
                        outs=[table1[t][0:N, :].rearrange(
                            "(k r) c -> k r c", k=NCORES).opt()])

            # ================= layer-1 edge phase (GD1=64) =================
            with tc.tile_pool(name="eg1", bufs=3) as gp, \
                 tc.tile_pool(name="ew1", bufs=3) as wp, \
                 tc.tile_pool(name="es1", bufs=4) as sp, \
                 tc.tile_pool(name="ef1", bufs=1) as fp, \
                 tc.tile_pool(name="park1", bufs=1) as parkp, \
                 tc.tile_pool(name="eps1", bufs=3, space="PSUM") as pp:

                gidx_sb = {}
                sdt = {}
                for t in "ab":
                    ncalls = sch1[t]["TCpad"] // NCH
                    gidx_sb[t] = fp.tile([128, ncalls, NCH * 8], I16,
                                         tag=f"g1x{t}", name=f"gidx1sb{t}")
                    nc.sync.dma_start(gidx_sb[t][:],
                                      gidx1_d[t].rearrange("c p s -> p c s"))
                    sdt[t] = fp.tile([128, NG1, 128], BF, tag=f"sdt1{t}",
                                     name=f"sdt1{t}")

                def sdt1_window(t, g):
                    """Slot-table gather for groups [g, g+NCH)."""
                    sdone = g * 128
                    n = min(NCH * 128, NG1 * 128 - sdone)
                    nc.gpsimd.dma_gather(
                        sdt[t][:, sdone // 128:(sdone + n) // 128, :],
                        table1[t][:, SC:SC + 128],
                        sdti1_sb[t][:, sdone // 16:(sdone + n) // 16],
                        n, n, 128, elem_step=TBC, queue_num=3)

                parks = {t: parkp.tile([128, NBLK, 128], F32, tag=f"park{t}",
                                       name=f"park1{t}") for t in "ab"}
                st = {t: dict(call=-1, G=None, pa=None) for t in "ab"}
                qctr = [0]
                NV = 128 // GD1   # 2 groups per 128-slot batch

                def scatter_acc(t, parks_t, acc, cols, mi):
                    """Fire pending scatter-adds once batches [c0, mi) done."""
                    cuts = {8: (0, 1024), 16: (8, 1024), NBLK: (16, 512)}
                    if mi in cuts:
                        c0, nI = cuts[mi]
                        nc.gpsimd.dma_scatter_add(
                            acc[:], parks_t[:, c0:c0 + nI // 128, :],
                            scat_sb[t][:, c0 * 8:c0 * 8 + nI // 16],
                            nI, nI, cols, queue_num=3)

                def do_group1(t, g):
                    sched = sch1[t]
                    cg = int(sched["Rg"][g] // (128 // GD1))
                    base = int(sched["cbase"][g])
                    s_ = st[t]
                    if g % NV == 0:
                        s_["pa"] = pp.tile([128, 136], F32, tag=f"pa{t}",
                                           name=f"pa1{t}")
                    pa = s_["pa"]
                    row0 = GD1 * (g % NV)
                    done = 0
                    while done < cg:
                        seg = min(NCH - (base + done) % NCH, cg - done)
                        call = (base + done) // NCH
                        coff = (base + done) % NCH
                        if call != s_["call"]:
                            G = gp.tile([128, NCH, TBC], BF, tag=f"G{t}",
                                        name=f"G1{t}")
                            nc.gpsimd.dma_gather(
                                G[:, :, :], table1[t][:],
                                gidx_sb[t][:, call, :],
                                NCH * 128, NCH * 128, TBC,
                                queue_num=qctr[0] % 4)
                            qctr[0] += 1
                            s_["call"] = call
                            s_["G"] = G
                        G = s_["G"]
                        sl = slice(coff, coff + seg)
                        u = sp.tile([128, NCH, 8], F32, tag=f"u{t}",
                                    name=f"u1{t}")
                        nc.vector.tensor_tensor(
                            out=u[:, :seg, :], in0=G[:, sl, SC:SC + 8],
                            in1=sdt[t][:, g, 8:16][:, None, :].to_broadcast(
                                [128, seg, 8]),
                            op=OP.add)
                        phi = sp.tile([128, NCH, 8], F32, tag=f"phi{t}",
                                      name=f"phi1{t}")
                        nc.vector.scalar_tensor_tensor(
                            out=phi[:, :seg, :], in0=u[:, :seg, :], scalar=NEG,
                            in1=u[:, :seg, :], op0=OP.mult, op1=OP.max)
                        q = sp.tile([128, NCH, 8], BF, tag=f"q{t}",
                                    name=f"q1{t}")
                        nc.scalar.activation(out=q[:, :seg, :],
                                             in_=phi[:, :seg, :], func=AF.Exp)
                        W = wp.tile([128, NCH, 136], BF, tag=f"W{t}",
                                    name=f"W1{t}")
                        nc.vector.tensor_tensor(
                            out=W[:, :seg, 0:128].rearrange(
                                "p s (h c) -> p s h c", h=8),
                            in0=G[:, sl, 0:128].rearrange(
                                "p s (h c) -> p s h c", h=8),
                            in1=q[:, :seg, :, None].to_broadcast(
                                [128, seg, 8, 16]),
                            op=OP.mult)
                        nc.scalar.copy(out=W[:, :seg, 128:136], in_=q[:, :seg, :])
                        for s in range(seg):
                            cc = done + s
                            nc.tensor.matmul(
                                out=pa[row0:row0 + GD1, :],
                                lhsT=sc1_sb[:], rhs=W[:, s, 0:136],
                                start=(cc == 0), stop=(cc == cg - 1),
                                skip_group_check=True)
                        done += seg
                    if g % NV == NV - 1:
                        mi = (g * GD1) // 128
                        z8 = sp.tile([128, 8], F32, tag=f"z8{t}",
                                     name=f"z81{t}")
                        nc.vector.tensor_scalar(
                            out=z8[:], in0=pa[:, 128:136], scalar1=1.0,
                            scalar2=1e-30, op0=OP.mult, op1=OP.max)
                        rz = sp.tile([128, 8], F32, tag=f"rz{t}",
                                     name=f"rz1{t}")
                        nc.vector.reciprocal(out=rz[:], in_=z8[:])
                        nc.vector.tensor_tensor(
                            out=parks[t][:, mi, :].rearrange(
                                "p (h c) -> p h c", h=8),
                            in0=pa[:, 0:128].rearrange(
                                "p (h c) -> p h c", h=8),
                            in1=rz[:].to_broadcast([128, 8, 16]),
                            op=OP.mult)
                        scatter_acc(t, parks[t], acc1, 128, mi + 1)

                for g in range(NG1):
                    for t in "ab":
                        if g % NCH == 0:
                            sdt1_window(t, g)
                        do_group1(t, g)

            # ---- combine acc1 -> h1' -> layer-2 table slice ----
            with tc.tile_pool(name="cmb", bufs=4) as cp, \
                 tc.tile_pool(name="cmbps", bufs=4, space="PSUM") as cps:
                for ti, t in enumerate("ab"):
                    for i in range(NBLK):
                        lo = i * 128
                        m = min(128, ND - lo)
                        a = cp.tile([128, 128], F32, tag="c_a")
                        nc.sync.dma_start(a[:m], acc1[lo:lo + m, :])
                        e = cp.tile([128, 128], F32, tag="c_e")
                        nc.scalar.activation(out=e[:m], in_=a[:m], func=AF.Exp,
                                             scale=0.5)
                        em1 = cp.tile([128, 128], F32, tag="c_em1")
                        nc.vector.tensor_scalar(out=em1[:m], in0=e[:m],
                                                scalar1=-1.0,
                                                scalar2=None, op0=OP.add)
                        xm = cp.tile([128, 128], F32, tag="c_xm")
                        nc.vector.tensor_scalar(out=xm[:m], in0=a[:m],
                                                scalar1=0.5,
                                                scalar2=None, op0=OP.mult)
                        mk = cp.tile([128, 128], mybir.dt.uint8, tag="c_mk")
                        nc.vector.tensor_scalar(out=mk[:m], in0=a[:m],
                                                scalar1=0.0,
                                                scalar2=None, op0=OP.is_gt)
                        h = cp.tile([128, 128], BF, tag="c_h")
                        nc.vector.select(out=h[:m], mask=mk[:m], on_true=xm[:m],
                                         on_false=em1[:m])
                        # transpose for the s2 projections
                        tps = cps.tile([128, 128], BF, tag="c_tp")
                        nc.tensor.transpose(out=tps[:, :m], in_=h[:m, :],
                                            identity=id_sb[:m, :m])
                        ht = cp.tile([128, 128], BF, tag="c_ht")
                        nc.scalar.copy(out=ht[:, :m], in_=tps[:, :m])
                        ps2 = cps.tile([128, 16], F32, tag="c_s2")
                        nc.tensor.matmul(out=ps2[:m], lhsT=ht[:, :m],
                                         rhs=w2aug[t][:, 512:528],
                                         start=True, stop=True)
                        o2 = cp.tile([128, 144], BF, tag="c_o2")
                        nc.vector.tensor_copy(out=o2[:m, 0:128], in_=h[:m])
                        nc.scalar.copy(out=o2[:m, 128:144], in_=ps2[:m])
                        nc.sync.dma_start(slice2[t][lo:lo + m, 0:144],
                                          o2[:m])
                    nc.gpsimd.collective_compute(
                        "AllGather", mybir.AluOpType.bypass,
                        replica_groups=RG,
                        ins=[slice2[t][:, :].opt()],
                        outs=[table2[t][0:N, :].rearrange(
                            "(k r) c -> k r c", k=NCORES).opt()])

            # ================= layer-2 edge phase (GD2=16) =================
            with tc.tile_pool(name="eg2", bufs=3) as gp2, \
                 tc.tile_pool(name="ew2", bufs=3) as wp2, \
                 tc.tile_pool(name="es2", bufs=4) as sp2, \
                 tc.tile_pool(name="ms2", bufs=2) as msp, \
                 tc.tile_pool(name="ef2", bufs=1) as fp2, \
                 tc.tile_pool(name="park2", bufs=1) as park2p, \
                 tc.tile_pool(name="sst2", bufs=2) as sstp, \
                 tc.tile_pool(name="emt2", bufs=3, space="PSUM") as mtp, \
                 tc.tile_pool(name="ezp2", bufs=1, space="PSUM") as zpp, \
                 tc.tile_pool(name="epp2", bufs=2, space="PSUM") as php:

                gidx2_sb = {}
                sdt2 = {}
                for t in "ab":
                    ncalls = sch2[t]["TCpad"] // NCH
                    gidx2_sb[t] = fp2.tile([128, ncalls, NCH * 8], I16,
                                           tag=f"g2x{t}", name=f"gidx2sb{t}")
                    nc.sync.dma_start(gidx2_sb[t][:],
                                      gidx2_d[t].rearrange("c p s -> p c s"))
                    sdt2[t] = fp2.tile([128, NG2, 8], BF, tag=f"sdt2{t}",
                                       name=f"sdt2{t}")

                def sdt2_window(t, g):
                    """Slot-table gather + s_dst compaction for [g, g+NCH)."""
                    sdone = g * 128
                    n = min(NCH * 128, NG2 * 128 - sdone)
                    stg = sstp.tile([128, NCH, 128], BF, tag=f"sst{t}",
                                    name=f"sst2{t}")
                    nc.gpsimd.dma_gather(
                        stg[:, 0:n // 128, :], table2[t][:, SC:SC + 128],
                        sdti2_sb[t][:, sdone // 16:(sdone + n) // 16],
                        n, n, 128, elem_step=TBC, queue_num=3)
                    nc.vector.tensor_copy(
                        out=sdt2[t][:, sdone // 128:(sdone + n) // 128, :],
                        in_=stg[:, 0:n // 128, 8:16])

                parks2 = {t: park2p.tile([128, NBLK, 64], F32, tag=f"park2{t}",
                                         name=f"park2{t}") for t in "ab"}
                st2 = {t: dict(call=-1, G=None, Q=None, zps=None, msb=None)
                       for t in "ab"}
                qctr2 = [0]
                NV2 = 128 // GD2   # 8 groups per 128-slot batch

                def scatter_acc2(t, mi):
                    cuts = {8: (0, 1024), 16: (8, 1024), NBLK: (16, 512)}
                    if mi in cuts:
                        c0, nI = cuts[mi]
                        nc.gpsimd.dma_scatter_add(
                            acc2[:], parks2[t][:, c0:c0 + nI // 128, :],
                            scat_sb[t][:, c0 * 8:c0 * 8 + nI // 16],
                            nI, nI, 64, queue_num=3)

                def do_group2(t, g):
                    sched = sch2[t]
                    cg = int(sched["Rg"][g] // (128 // GD2))
                    base = int(sched["cbase"][g])
                    s_ = st2[t]
                    if g % NV2 == 0:
                        s_["zps"] = zpp.tile([8, 128], F32, tag=f"zp{t}",
                                             name=f"zps2{t}")
                        # [c1, h, grp*16+d]: per-head slice is 2D-contiguous
                        s_["msb"] = msp.tile([128, 8, 128], BF, tag=f"ms{t}",
                                             name=f"msb2{t}")
                    zps, msb = s_["zps"], s_["msb"]
                    mt = mtp.tile([128, 128], F32, tag="mt", name=f"mt2{t}")
                    zcol = GD2 * (g % NV2)
                    done = 0
                    while done < cg:
                        seg = min(NCH - (base + done) % NCH, cg - done)
                        call = (base + done) // NCH
                        coff = (base + done) % NCH
                        if call != s_["call"]:
                            G = gp2.tile([128, NCH, TBC], BF, tag=f"G{t}",
                                         name=f"G2{t}")
                            nc.gpsimd.dma_gather(
                                G[:, :, :], table2[t][:],
                                gidx2_sb[t][:, call, :],
                                NCH * 128, NCH * 128, TBC,
                                queue_num=qctr2[0] % 4)
                            qctr2[0] += 1
                            s_["call"] = call
                            s_["G"] = G
                        G = s_["G"]
                        sl = slice(coff, coff + seg)
                        u = sp2.tile([128, NCH, 8], F32, tag=f"u{t}",
                                     name=f"u2{t}")
                        nc.vector.tensor_tensor(
                            out=u[:, :seg, :], in0=G[:, sl, SC:SC + 8],
                            in1=sdt2[t][:, g, :][:, None, :].to_broadcast(
                                [128, seg, 8]),
                            op=OP.add)
                        phi = sp2.tile([128, NCH, 8], F32, tag=f"phi{t}",
                                       name=f"phi2{t}")
                        nc.vector.scalar_tensor_tensor(
                            out=phi[:, :seg, :], in0=u[:, :seg, :], scalar=NEG,
                            in1=u[:, :seg, :], op0=OP.mult, op1=OP.max)
                        q = sp2.tile([128, NCH, 8], BF, tag=f"q{t}",
                                     name=f"q2{t}")
                        nc.scalar.activation(out=q[:, :seg, :],
                                             in_=phi[:, :seg, :], func=AF.Exp)
                        Q = wp2.tile([128, NCH, 128], BF, tag=f"Q{t}",
                                     name=f"Q2{t}")
                        nc.vector.tensor_tensor(
                            out=Q[:, :seg, :].rearrange(
                                "p s (h d) -> p s h d", h=8),
                            in0=q[:, :seg, :, None].to_broadcast(
                                [128, seg, 8, GD2]),
                            in1=sc2_sb[:, None, None, :].to_broadcast(
                                [128, seg, 8, GD2]),
                            op=OP.mult)
                        for s in range(seg):
                            cc = done + s
                            first = cc == 0
                            last = cc == cg - 1
                            nc.tensor.matmul(
                                out=mt[:], lhsT=G[:, coff + s, 0:128],
                                rhs=Q[:, s, :],
                                start=first, stop=last,
                                skip_group_check=True)
                            nc.tensor.matmul(
                                out=zps[:, zcol:zcol + GD2],
                                lhsT=q[:, s, :], rhs=sc2_sb[:],
                                start=first, stop=last,
                                skip_group_check=True)
                        done += seg
                    # park M_T for this group (cast f32 psum -> bf16 sbuf)
                    gb = (g % NV2) * GD2
                    nc.vector.tensor_copy(
                        out=msb[:, :, gb:gb + GD2],
                        in_=mt[:].rearrange("p (h d) -> p h d", h=8))
                    if g % NV2 == NV2 - 1:
                        mi = (g * GD2) // 128
                        zsb = sp2.tile([8, 128], BF, tag=f"zs{t}",
                                       name=f"zsb2{t}")
                        nc.vector.tensor_copy(out=zsb[:], in_=zps[:])
                        ztp = zpp.tile([128, 8], BF, tag="zt",
                                       name=f"ztp2{t}")
                        nc.tensor.transpose(out=ztp[:], in_=zsb[:],
                                            identity=id_sb[0:8, 0:8])
                        z8 = sp2.tile([128, 8], F32, tag=f"z8{t}",
                                      name=f"z82{t}")
                        nc.vector.tensor_scalar(
                            out=z8[:], in0=ztp[:], scalar1=8.0,
                            scalar2=1e-30, op0=OP.mult, op1=OP.max)
                        rz = sp2.tile([128, 8], F32, tag=f"rz{t}",
                                      name=f"rz2{t}")
                        nc.vector.reciprocal(out=rz[:], in_=z8[:])
                        pv = php.tile([128, 8, 64], F32, tag="pv",
                                      name=f"pv2{t}")
                        for h in range(8):
                            nc.tensor.matmul(
                                out=pv[:, h, :],
                                lhsT=msb[:, h, :],
                                rhs=w2aug[t][:, h * 64:(h + 1) * 64],
                                start=True, stop=True,
                                skip_group_check=True)
                        tmp = sp2.tile([128, 8, 64], F32, tag=f"tmp{t}",
                                       name=f"tmp2{t}")
                        nc.vector.tensor_tensor(
                            out=tmp[:], in0=pv[:],
                            in1=rz[:, :, None].to_broadcast([128, 8, 64]),
                            op=OP.mult)
                        nc.vector.tensor_reduce(
                            out=parks2[t][:, mi, :, None],
                            in_=tmp[:].rearrange("p h c -> p c h"),
                            axis=AX.X, op=OP.add)
                        scatter_acc2(t, mi + 1)

                for g in range(NG2):
                    for t in "ab":
                        if g % NCH == 0:
                            sdt2_window(t, g)
                        do_group2(t, g)

            # ---- classifier ----
            with tc.tile_pool(name="cls", bufs=4) as clsp, \
                 tc.tile_pool(name="clsps", bufs=2, space="PSUM") as clsps:
                for i in range(NBLK):
                    lo = i * 128
                    m = min(128, ND - lo)
                    a = clsp.tile([128, 64], F32, tag="k_a")
                    nc.sync.dma_start(a[:m], acc2[lo:lo + m, :])
                    e = clsp.tile([128, 64], F32, tag="k_e")
                    nc.scalar.activation(out=e[:m], in_=a[:m], func=AF.Exp,
                                         scale=0.5)
                    em1 = clsp.tile([128, 64], F32, tag="k_em1")
                    nc.vector.tensor_scalar(out=em1[:m], in0=e[:m], scalar1=-1.0,
                                            scalar2=None, op0=OP.add)
                    xm = clsp.tile([128, 64], F32, tag="k_xm")
                    nc.vector.tensor_scalar(out=xm[:m], in0=a[:m], scalar1=0.5,
                                            scalar2=None, op0=OP.mult)
                    mk = clsp.tile([128, 64], mybir.dt.uint8, tag="k_mk")
                    nc.vector.tensor_scalar(out=mk[:m], in0=a[:m], scalar1=0.0,
                                            scalar2=None, op0=OP.is_gt)
                    h = clsp.tile([128, 64], BF, tag="k_h")
                    nc.vector.select(out=h[:m], mask=mk[:m], on_true=xm[:m],
                                     on_false=em1[:m])
                    tps = clsps.tile([64, 128], BF, tag="k_t1")
                    nc.tensor.transpose(out=tps[:, :m], in_=h[:m, :],
                                        identity=id_sb[:m, :m])
                    h3t = clsp.tile([64, 128], BF, tag="k_h3t")
                    nc.scalar.copy(out=h3t[:, :m], in_=tps[:, :m])
                    z1 = clsps.tile([128, 32], F32, tag="k_z1")
                    nc.tensor.matmul(out=z1[:m], lhsT=h3t[:, :m], rhs=wc1_sb[:],
                                     start=True, stop=True)
                    z1s = clsp.tile([128, 32], BF, tag="k_z1s")
                    nc.scalar.activation(out=z1s[:m], in_=z1[:m], func=AF.Relu)
                    t2ps = clsps.tile([32, 128], BF, tag="k_t2")
                    nc.tensor.transpose(out=t2ps[:, :m], in_=z1s[:m, :],
                                        identity=id_sb[:m, :m])
                    z1t = clsp.tile([32, 128], BF, tag="k_z1t")
                    nc.scalar.copy(out=z1t[:, :m], in_=t2ps[:, :m])
                    lg = clsps.tile([128, 2], F32, tag="k_lg")
                    nc.tensor.matmul(out=lg[:m], lhsT=z1t[:, :m], rhs=wc2_sb[:],
                                     start=True, stop=True)
                    lo_ = clsp.tile([128, 2], F32, tag="k_out")
                    nc.vector.tensor_copy(out=lo_[:m], in_=lg[:m])
                    nc.sync.dma_start(out[lo:lo + m, :], lo_[:m])

    nc.compile()
    return nc


# ----------------------------------------------------------------------------
# entry point
# ----------------------------------------------------------------------------

_CACHE = {}


def _prepare(inputs):
    per_core, sch1, sch2 = _host_prep(inputs)
    key = (tuple(sch1[t]["TCpad"] for t in "ab"),
           tuple(sch2[t]["TCpad"] for t in "ab"),
           tuple(tuple(sch1[t]["Rg"]) for t in "ab"),
           tuple(tuple(sch2[t]["Rg"]) for t in "ab"))
    if key not in _CACHE:
        _CACHE.clear()
        _CACHE[key] = _build_nc(sch1, sch2)
    return _CACHE[key], per_core


def _run(nc, per_core, **kw):
    from concourse import bass_utils
    return bass_utils.run_bass_kernel_spmd(nc, per_core,
                                           core_ids=list(range(NCORES)), **kw)


def kernel(**inputs):
    nc, per_core = _prepare(inputs)
    res = _run(nc, per_core)
    return np.concatenate([res.results[k]["out"] for k in range(NCORES)], 0)
